# revision 16
# baseline (speedup 1.0000x reference)
import numpy as np

# nn_MyTemporalAttention: x [32, 64, 2048] -> y [32, 2048, 64]
B, C, L = 32, 64, 2048
KQ = 50
NCORES = 8
BPC = B // NCORES          # batches per core
NM = L // 128              # m-chunks of 128

TRACE = False
LAST_EXEC_NS = None
_cache = {}


def _build(scale: float):
    import concourse.bass as bass
    import concourse.tile as tile
    from concourse import bacc, mybir
    from contextlib import ExitStack

    FP32 = mybir.dt.float32
    BF16 = mybir.dt.bfloat16
    I32 = mybir.dt.int32
    AF = mybir.ActivationFunctionType
    OP = mybir.AluOpType
    ts = bass.ts

    # Schraudolph exp: bits = int32(SA*x + SB); valid since elu input <= 0
    SA = float(2**23 / np.log(2))
    SB = float(127 * 2**23 - 486408)

    nc = bacc.Bacc(
        "TRN2",
        target_bir_lowering=False,
        debug=False,
        enable_asserts=False,
        num_devices=NCORES,
    )
    # x host-augmented with ones row, bf16: [BPC, 65, L]
    x_d = nc.dram_tensor("x", [BPC, C + 1, L], BF16, kind="ExternalInput").ap()
    # wkq: [65, 128], cols 0-49 = [Wk;bk], cols 64-113 = [Wq;bq], rest zero
    wkq_d = nc.dram_tensor("wkq", [C + 1, 128], BF16, kind="ExternalInput").ap()
    # wv = [Wv; bv] -> [65, 64]
    wv_d = nc.dram_tensor("wv", [C + 1, C], BF16, kind="ExternalInput").ap()
    # y packed: [BPC, 2, 64, 1024]; [b, h, c, j] = y[b, h*1024+j, c]
    y_d = nc.dram_tensor("y", [BPC, 2, C, 1024], FP32, kind="ExternalOutput").ap()

    with tile.TileContext(nc) as tc, ExitStack() as ctx:
        const = ctx.enter_context(tc.tile_pool(name="const", bufs=1))
        xpool = ctx.enter_context(tc.tile_pool(name="xp", bufs=BPC))
        kqpool = ctx.enter_context(tc.tile_pool(name="kqt", bufs=2))
        kq2pool = ctx.enter_context(tc.tile_pool(name="kq2", bufs=2))
        xmpool = ctx.enter_context(tc.tile_pool(name="xm", bufs=2))
        xepool = ctx.enter_context(tc.tile_pool(name="xe", bufs=2))
        vpool = ctx.enter_context(tc.tile_pool(name="v", bufs=2))
        epool = ctx.enter_context(tc.tile_pool(name="e", bufs=4))
        ytpool = ctx.enter_context(tc.tile_pool(name="yt", bufs=2))
        spool = ctx.enter_context(tc.tile_pool(name="stats", bufs=8))
        vppool = ctx.enter_context(tc.tile_pool(name="vp", bufs=4))
        # PSUM: pw 3 x [128,1024] f32 (6 banks) + py 1 x [128,1024] f32 (2)
        pw = ctx.enter_context(tc.tile_pool(name="pw", bufs=3, space="PSUM"))
        py = ctx.enter_context(tc.tile_pool(name="py", bufs=1, space="PSUM"))

        wkq = const.tile([C + 1, 128], BF16)
        nc.sync.dma_start(wkq[:], wkq_d[:])
        wv = const.tile([C + 1, C], BF16)
        nc.sync.dma_start(wv[:], wv_d[:])

        # trigger the exp table-set load early so it overlaps the prologue
        warm = const.tile([1, 2], FP32)
        nc.vector.memset(warm[0:1, 0:1], 0.0)
        nc.scalar.activation(warm[0:1, 1:2], warm[0:1, 0:1], AF.Exp)

        xps = []
        for b in range(BPC):
            xp = xpool.tile([C + 1, L], BF16)
            nc.sync.dma_start(xp[:], x_d[b])
            xps.append(xp)

        def prep_tasks(b):
            """Emission closures producing kqt/kq2/vsb for batch b."""
            xp = xps[b]
            # kqt: k rows at partitions 0-49, q rows at partitions 64-113
            # kq2 (mirror): q rows at partitions 0-49, k rows at 64-113
            kqt = kqpool.tile([128, L], BF16)
            kq2 = kq2pool.tile([128, L], BF16)
            vsb = vpool.tile([128, NM * C], BF16)
            pkqs = [None, None]

            def kq_mm(h):
                pkq = pw.tile([128, 1024], FP32, name="pwm")
                pkqs[h] = pkq
                for j in range(2):
                    nc.tensor.matmul(
                        pkq[:, ts(j, 512)], wkq[:], xp[:, ts(2 * h + j, 512)],
                        start=True, stop=True,
                    )

            def elu(h):
                pkq = pkqs[h]
                xm = xmpool.tile([128, 1024], BF16)
                nc.vector.tensor_scalar_min(xm[:], pkq[:], 0.0)
                xe = xepool.tile([128, 1024], I32)
                nc.vector.tensor_scalar(xe[:], xm[:], SA, SB, OP.mult, OP.add)
                nc.vector.scalar_tensor_tensor(
                    kqt[:, ts(h, 1024)], xe[:].bitcast(FP32), -1.0, pkq[:],
                    OP.add, OP.max,
                )

            def qdup(h):
                sl = slice(1024 * h, 1024 * (h + 1))
                nc.sync.dma_start(kq2[0:KQ, sl], kqt[64 : 64 + KQ, sl])
                nc.sync.dma_start(kq2[64 : 64 + KQ, sl], kqt[0:KQ, sl])

            pvs = [None]

            def v_mm(part):
                if part == 0:
                    pvs[0] = pw.tile([128, 1024], FP32, name="pwm")
                pv = pvs[0]
                for jj in range(4 * part, 4 * part + 4):
                    nc.tensor.matmul(
                        pv[:, ts(jj, C)], xp[:, ts(jj, 128)], wv[:],
                        start=True, stop=True,
                    )

            def v_tanh():
                nc.scalar.activation(vsb[:], pvs[0][:], AF.Tanh)

            # first 6 tasks are the critical chain to the first exp of the
            # batch; the rest can lag
            tasks = [
                lambda: kq_mm(0),
                lambda: elu(0),
                lambda: qdup(0),
                lambda: kq_mm(1),
                lambda: elu(1),
                lambda: qdup(1),
            ]
            for part in range(4):
                tasks.append(lambda part=part: v_mm(part))
            tasks.append(v_tanh)
            return kqt, kq2, vsb, tasks

        kqt, kq2, vsb, tasks0 = prep_tasks(0)
        for t in tasks0[:6]:
            t()

        for b in range(BPC):
            if b + 1 < BPC:
                kqt_n, kq2_n, vsb_n, tasks = prep_tasks(b + 1)
            else:
                tasks = []

            pyt = py.tile([128, 1024], FP32, name="pyt")

            def emit_mm2(m):
                # Two concurrent row-tiled streams: A on PE rows 0-49
                # (l-half 0), B on rows 64-113 (l-half 1).
                tiles = [
                    pw.tile([128, 1024], FP32, name="pwm"),
                    pw.tile([128, 1024], FP32, name="pwm"),
                ]
                for jj in range(2):
                    nc.tensor.matmul(
                        tiles[0][:, ts(jj, 512)],
                        kq2[0:KQ, ts(m, 128)],
                        kqt[0:KQ, ts(jj, 512)],
                        start=True,
                        stop=True,
                    )
                    nc.tensor.matmul(
                        tiles[1][:, ts(jj, 512)],
                        kqt[64 : 64 + KQ, ts(m, 128)],
                        kq2[64 : 64 + KQ, ts(2 + jj, 512)],
                        start=True,
                        stop=True,
                    )
                return tiles

            pw2 = emit_mm2(0)
            if b == 0:
                # batch-0 v/tanh: after mm2(0) so the first exps aren't
                # queued behind them, but before the loop (vsb needed at m=0)
                for t in tasks0[6:]:
                    t()
            ti = 0
            for m in range(NM):
                d2 = [spool.tile([128, 1], FP32, name=f"d2{h}") for h in range(2)]
                et = epool.tile([128, L], BF16)
                for h in range(2):
                    nc.scalar.activation(
                        et[:, ts(h, 1024)], pw2[h][:], AF.Exp, scale=scale,
                        accum_out=d2[h][:],
                    )
                if m + 1 < NM:
                    pw2 = emit_mm2(m + 1)
                dsum = spool.tile([128, 1], FP32)
                nc.vector.tensor_add(dsum[:], d2[0][:], d2[1][:])
                dinv = spool.tile([128, 1], FP32)
                nc.vector.reciprocal(dinv[:], dsum[:])
                vp = vppool.tile([128, C], BF16)
                nc.vector.tensor_scalar_mul(vp[:], vsb[:, ts(m, C)], dinv[:])
                # y^T accum: quadrant (j//2) selects partition half; order
                # 0,2,1,3 pairs different col-groups for concurrency
                for j in (0, 2, 1, 3):
                    nc.tensor.matmul(
                        pyt[64 * (j // 2) : 64 * (j // 2) + C, ts(j % 2, 512)],
                        vp[:],
                        et[:, ts(j, 512)],
                        start=(m == 0),
                        stop=(m == NM - 1),
                    )
                if ti < len(tasks) and m >= 2:
                    tasks[ti]()
                    ti += 1

            while ti < len(tasks):
                tasks[ti]()
                ti += 1

            yt = ytpool.tile([128, 1024], FP32)
            nc.vector.tensor_copy(yt[:], pyt[:])
            nc.sync.dma_start(y_d[b][0], yt[0:C, :])
            nc.sync.dma_start(y_d[b][1], yt[64 : 64 + C, :])

            if b + 1 < BPC:
                kqt, kq2, vsb = kqt_n, kq2_n, vsb_n

    nc.finalize()
    return nc


def kernel(x, Wk, bk, Wq, bq, Wv, bv, sample_len):
    global LAST_EXEC_NS
    from concourse.bass_utils import run_bass_kernel_spmd

    scale = float(1.0 / np.sqrt(np.float64(sample_len)))
    if scale not in _cache:
        _cache[scale] = _build(scale)
    nc = _cache[scale]

    import ml_dtypes

    bf16 = ml_dtypes.bfloat16
    x = np.asarray(x, dtype=np.float32)
    ones = np.ones((B, 1, L), dtype=np.float32)
    x = np.ascontiguousarray(np.concatenate([x, ones], axis=1)).astype(bf16)
    wkq = np.zeros((C + 1, 128), dtype=np.float32)
    wkq[:, 0:KQ] = np.concatenate([Wk, bk[None, :]], axis=0)
    wkq[:, 64 : 64 + KQ] = np.concatenate([Wq, bq[None, :]], axis=0)
    wkq = wkq.astype(bf16)
    wv = np.concatenate([Wv, bv[None, :]], axis=0).astype(bf16)

    in_maps = [
        {"x": x[i * BPC : (i + 1) * BPC], "wkq": wkq, "wv": wv}
        for i in range(NCORES)
    ]
    res = run_bass_kernel_spmd(nc, in_maps, list(range(NCORES)), trace=TRACE)
    LAST_EXEC_NS = res.exec_time_ns
    yp = np.concatenate([res.results[i]["y"] for i in range(NCORES)], axis=0)
    # yp: [B, 2, 64, 1024] -> y: [B, 2048, 64]
    y = yp.transpose(0, 1, 3, 2).reshape(B, L, C)
    return np.ascontiguousarray(y)
